# revision 1
# baseline (speedup 1.0000x reference)
"""GQA attention layer (B=2, S=2048, D=4096, 32 Q heads / 8 KV heads, RoPE,
causal) on 8 Trainium2 NeuronCores, tensor-parallel over heads.

Each core owns 4 Q heads + 1 KV head: it computes its Q/K/V projections,
RoPE, causal attention, and a partial output projection (rank-512 slice of
the wo contraction).  The host sums the 8 partial outputs.

Layouts are feature-major ("transposed") on chip: activations live as
[feature_partition, token_free] so every matmul contracts over the
partition dim with wide (>=256) moving operands, keeping the PE at full
rate with float32r (fp22-precision fp32) operands.
"""

import os
import sys
import types
from contextlib import ExitStack

import numpy as np

import concourse.bass as bass
import concourse.tile as tile
from concourse import bacc
from concourse import mybir
from concourse import bass_utils
from concourse.bass_utils import run_bass_kernel_spmd

# ---------------------------------------------------------------------------
# Optional NTFF profiling support under axon. The trimmed image's `antenv`
# lacks `axon_hooks`, so run_bass_kernel_spmd(trace=True) would silently skip
# tracing; register the hook ourselves. Harmless when unavailable.
try:
    import antenv  # noqa: F401
    from trn_agent_boot.trn_boot import _ntff_profile_via_ctypes

    if "antenv.axon_hooks" not in sys.modules:
        _hooks_mod = types.ModuleType("antenv.axon_hooks")
        _hook = _ntff_profile_via_ctypes("/opt/axon/libaxon_pjrt.so")
        _hooks_mod.get_axon_ntff_profile_hook = lambda: _hook
        _hooks_mod.set_axon_ntff_profile_hook = lambda h: None
        sys.modules["antenv.axon_hooks"] = _hooks_mod
    bass_utils.upload_artifacts = lambda tmpdir: "local://skipped"
except Exception:
    pass

F32 = mybir.dt.float32
F32R = mybir.dt.float32r
EXP = mybir.ActivationFunctionType.Exp

B, S, D = 2, 2048, 4096
NH, NKV, HD = 32, 8, 128
T = B * S                       # 4096 tokens total
N_CORES = 8
QH = NH // N_CORES              # 4 local q heads
FL = QH * HD                    # 512 local q features
SCALE = 1.0 / float(np.sqrt(HD))
NEG = -1.0e30

NW = 512                        # token-group width in the QKV projection
QT = 256                        # q-token group width in attention (AV moving dim)
DKD = D // 128                  # 32 contraction chunks for projections


def _build_program():
    nc = bacc.Bacc("TRN2", target_bir_lowering=False, debug=False,
                   num_devices=N_CORES)

    xT = nc.dram_tensor("xT", [D, T], F32R, kind="ExternalInput").ap()
    wqT = nc.dram_tensor("wqT", [D, FL], F32R, kind="ExternalInput").ap()
    wkT = nc.dram_tensor("wkT", [D, HD], F32R, kind="ExternalInput").ap()
    wvT = nc.dram_tensor("wvT", [D, HD], F32R, kind="ExternalInput").ap()
    woT = nc.dram_tensor("woT", [FL, D], F32R, kind="ExternalInput").ap()
    # RoPE constants, pre-assembled for the rotate-half formulation on the
    # even/odd-split feature layout: ropc = [cos; cos], rops = [-sin; sin].
    ropc = nc.dram_tensor("ropc", [HD, S], F32, kind="ExternalInput").ap()
    rops = nc.dram_tensor("rops", [HD, S], F32, kind="ExternalInput").ap()
    idin = nc.dram_tensor("idin", [128, 128], F32R, kind="ExternalInput").ap()
    onesin = nc.dram_tensor("onesin", [128, 1], F32R, kind="ExternalInput").ap()
    maskt = [nc.dram_tensor(f"maskt{v}", [128, 4 * 128], F32,
                            kind="ExternalInput").ap() for v in range(4)]
    y = nc.dram_tensor("y", [T, D], F32, kind="ExternalOutput").ap()

    with tile.TileContext(nc) as tc, ExitStack() as ctx:
        dram = ctx.enter_context(tc.tile_pool(name="dram", bufs=1, space="DRAM"))
        qT_d = [dram.tile([FL, S], F32R, tag=f"qT_d{b}", name=f"qT_d{b}")
                for b in range(B)]
        kT_d = [dram.tile([HD, S], F32R, tag=f"kT_d{b}", name=f"kT_d{b}")
                for b in range(B)]
        vT_d = [dram.tile([HD, S], F32R, tag=f"vT_d{b}", name=f"vT_d{b}")
                for b in range(B)]

        const = ctx.enter_context(tc.tile_pool(name="const", bufs=1))
        ident = const.tile([128, 128], F32R)
        nc.sync.dma_start(ident[:], idin)
        ones_t = const.tile([128, 1], F32R)
        nc.sync.dma_start(ones_t[:], onesin)
        mtv = []
        for v in range(4):
            mt = const.tile([128, 4 * 128], F32, tag=f"mtv{v}", name=f"mtv{v}")
            nc.sync.dma_start(mt[:], maskt[v])
            mtv.append(mt)

        # ------------------------------------------------------------------
        # Phase 1: QKV projections + RoPE  ->  DRAM scratch (feature-major)
        # ------------------------------------------------------------------
        with tc.tile_pool(name="wqkv", bufs=1) as wpool, \
             tc.tile_pool(name="ropec", bufs=1) as rcpool, \
             tc.tile_pool(name="xin", bufs=3) as xpool, \
             tc.tile_pool(name="qkvstage", bufs=2) as stage, \
             tc.tile_pool(name="ropetmp", bufs=2) as rtmp, \
             tc.tile_pool(name="qkvps", bufs=1, space="PSUM") as qkvps:

            cos_s = rcpool.tile([HD, S], F32)
            nc.sync.dma_start(cos_s[:], ropc)
            sin_s = rcpool.tile([HD, S], F32)
            nc.sync.dma_start(sin_s[:], rops)

            # Resident weights, packed k-chunk-major: [128, DKD * width]
            wq_sb = wpool.tile([128, DKD * FL], F32R, tag="wq")
            nc.sync.dma_start(
                wq_sb[:].rearrange("p (k f) -> p k f", k=DKD),
                wqT.rearrange("(k p) f -> p k f", p=128))
            wk_sb = wpool.tile([128, DKD * HD], F32R, tag="wk")
            nc.sync.dma_start(
                wk_sb[:].rearrange("p (k f) -> p k f", k=DKD),
                wkT.rearrange("(k p) f -> p k f", p=128))
            wv_sb = wpool.tile([128, DKD * HD], F32R, tag="wv")
            nc.sync.dma_start(
                wv_sb[:].rearrange("p (k f) -> p k f", k=DKD),
                wvT.rearrange("(k p) f -> p k f", p=128))

            def rope_evict(ps, out_sb, pos0, use_dve=False):
                """out_sb = RoPE(ps) on the even/odd-split feature layout
                (partitions 0..63 even pair components, 64..127 odd):
                out = x * [c;c] + swap_halves(x) * [-s;s]."""
                c = cos_s[:, pos0:pos0 + NW]
                s = sin_s[:, pos0:pos0 + NW]
                xsb = rtmp.tile([128, NW], F32, tag="xsb")
                if use_dve:
                    nc.vector.tensor_copy(xsb[:], ps[:])
                else:
                    nc.scalar.copy(xsb[:], ps[:])
                xsw = rtmp.tile([128, NW], F32, tag="xsw")
                nc.sync.dma_start(xsw[0:64, :], xsb[64:128, :])
                nc.sync.dma_start(xsw[64:128, :], xsb[0:64, :])
                t1 = rtmp.tile([128, NW], F32, tag="t1")
                nc.vector.tensor_mul(t1[:], xsw[:], s)
                nc.vector.tensor_mul(out_sb[:], xsb[:], c)
                nc.vector.tensor_add(out_sb[:], out_sb[:], t1[:])

            for n in range(T // NW):
                pos0 = (n * NW) % S
                qps = [qkvps.tile([128, NW], F32, tag=f"qps{m}", name=f"qps{m}")
                       for m in range(QH)]
                kps = qkvps.tile([128, NW], F32, tag="kps")
                vps = qkvps.tile([128, NW], F32, tag="vps")
                for k in range(DKD):
                    xt = xpool.tile([128, NW], F32R)
                    nc.sync.dma_start(
                        xt[:], xT[k * 128:(k + 1) * 128, n * NW:(n + 1) * NW])
                    st = (k == 0)
                    sp = (k == DKD - 1)
                    for m in range(QH):
                        nc.tensor.matmul(
                            qps[m][:],
                            wq_sb[:, k * FL + m * 128:k * FL + (m + 1) * 128],
                            xt[:], start=st, stop=sp)
                    nc.tensor.matmul(
                        kps[:], wk_sb[:, k * HD:(k + 1) * HD], xt[:],
                        start=st, stop=sp)
                    nc.tensor.matmul(
                        vps[:], wv_sb[:, k * HD:(k + 1) * HD], xt[:],
                        start=st, stop=sp)
                for m in range(QH):
                    qst = stage.tile([128, NW], F32R, tag=f"qst{m}", name=f"qst{m}")
                    rope_evict(qps[m], qst, pos0, use_dve=(m % 2 == 1))
                    nc.sync.dma_start(
                        qT_d[n * NW // S][m * 128:(m + 1) * 128,
                                          (n * NW) % S:(n * NW) % S + NW], qst[:])
                kst = stage.tile([128, NW], F32R, tag="kst")
                rope_evict(kps, kst, pos0)
                nc.sync.dma_start(
                    kT_d[n * NW // S][:, (n * NW) % S:(n * NW) % S + NW], kst[:])
                vst = stage.tile([128, NW], F32R, tag="vst")
                nc.vector.tensor_copy(vst[:], vps[:])
                nc.sync.dma_start(
                    vT_d[n * NW // S][:, (n * NW) % S:(n * NW) % S + NW], vst[:])

        # ------------------------------------------------------------------
        # Phase 2: attention + output projection
        # ------------------------------------------------------------------
        with tc.tile_pool(name="wo", bufs=1) as wopool, \
             tc.tile_pool(name="kv", bufs=2) as kvpool, \
             tc.tile_pool(name="qheads", bufs=1) as qpool, \
             tc.tile_pool(name="ptiles", bufs=4) as ptpool, \
             tc.tile_pool(name="attn", bufs=2) as atpool, \
             tc.tile_pool(name="smax", bufs=2) as smpool, \
             tc.tile_pool(name="ystage", bufs=2) as ypool, \
             tc.tile_pool(name="sps", bufs=2, space="PSUM") as spsum, \
             tc.tile_pool(name="vtps", bufs=1, space="PSUM") as vtpsum, \
             tc.tile_pool(name="sums", bufs=1, space="PSUM") as smpsum, \
             tc.tile_pool(name="avps", bufs=2, space="PSUM") as avpsum, \
             tc.tile_pool(name="yps", bufs=2, space="PSUM") as ypsum:

            QB = 512                       # q-block width in attention
            wo_sb = wopool.tile([128, QH * D], F32R)
            nc.sync.dma_start(
                wo_sb[:].rearrange("p (f d) -> p f d", f=QH),
                woT.rearrange("(f p) d -> p f d", p=128))

            for b in range(B):
                t0 = b * S
                ktb = kvpool.tile([128, S], F32R, tag="ktb")
                nc.sync.dma_start(ktb[:], kT_d[b][:, :])
                # V token-major: V_b[:, kc*128:+128] = vT[:, kc-block].T
                vtb = kvpool.tile([128, S], F32R, tag="vtb")
                nc.sync.dma_start(vtb[:], vT_d[b][:, :])
                V_b = kvpool.tile([128, S], F32R, tag="V_b")
                for kc in range(S // 128):
                    vt_ps = vtpsum.tile([128, 128], F32R)
                    nc.tensor.transpose(
                        vt_ps[:], vtb[:, kc * 128:(kc + 1) * 128], ident[:])
                    nc.vector.tensor_copy(
                        V_b[:, kc * 128:(kc + 1) * 128], vt_ps[:])

                qtb = [qpool.tile([128, S], F32R, tag=f"qtb{h}", name=f"qtb{h}")
                       for h in range(QH)]
                for h in range(QH):
                    nc.sync.dma_start(
                        qtb[h][:], qT_d[b][h * 128:(h + 1) * 128, :])

                def emit_wo(att_prev, q0_prev):
                    for tcx in range(QB // 128):
                        tg0 = t0 + q0_prev + tcx * 128
                        for dg in range(D // NW):
                            yp = ypsum.tile([128, NW], F32)
                            for f in range(QH):
                                nc.tensor.matmul(
                                    yp[:],
                                    att_prev[f][:, tcx * 128:(tcx + 1) * 128],
                                    wo_sb[:, f * D + dg * NW:f * D + (dg + 1) * NW],
                                    start=(f == 0), stop=(f == QH - 1))
                            ysb = ypool.tile([128, NW], F32)
                            nc.scalar.copy(ysb[:], yp[:])
                            nc.sync.dma_start(
                                y[tg0:tg0 + 128, dg * NW:(dg + 1) * NW], ysb[:])

                pending = None
                for qb in range(S // QB):
                    nkt = (qb + 1) * (QB // 128)     # causal 128-wide kt chunks
                    q0 = qb * QB
                    att = [atpool.tile([128, QB], F32R, tag=f"att{h}",
                                       name=f"att{h}") for h in range(QH)]
                    for h in range(QH):
                        # S.T = k.T-stationary @ q-moving: [kt, q]; exp
                        # straight from PSUM (no max subtraction: |scale*S|
                        # is small); causal mask added on the diagonal
                        # chunks; row sums via a ones-column matmul.
                        avp = avpsum.tile([128, QB], F32)
                        smp = smpsum.tile([1, QB], F32)
                        for ktc in range(nkt):
                            stp = spsum.tile([128, QB], F32)
                            nc.tensor.matmul(
                                stp[:], ktb[:, ktc * 128:(ktc + 1) * 128],
                                qtb[h][:, q0:q0 + QB], start=True, stop=True)
                            if ktc >= nkt - 4:
                                nc.vector.tensor_add(
                                    stp[:], stp[:], mtv[ktc - (nkt - 4)][:])
                            pt = ptpool.tile([128, QB], F32R)
                            nc.scalar.activation(pt[:], stp[:], EXP, scale=SCALE)
                            nc.tensor.matmul(
                                avp[:], V_b[:, ktc * 128:(ktc + 1) * 128],
                                pt[:], start=(ktc == 0), stop=(ktc == nkt - 1))
                            nc.tensor.matmul(
                                smp[:], ones_t[:], pt[:],
                                start=(ktc == 0), stop=(ktc == nkt - 1))
                        # Fast PSUM eviction (ACT copies), then normalize off
                        # the critical path: att = att_un * (1/sums).
                        attu = atpool.tile([128, QB], F32, tag=f"attu{h}",
                                           name=f"attu{h}", bufs=1)
                        nc.scalar.copy(attu[:], avp[:])
                        s_sb = smpool.tile([1, QB], F32, tag="s_sb")
                        nc.scalar.copy(s_sb[:], smp[:])
                        r_sb = smpool.tile([1, QB], F32, tag="r_sb")
                        nc.vector.reciprocal(r_sb[:], s_sb[:])
                        r_bc = smpool.tile([128, QB], F32, tag="r_bc")
                        nc.gpsimd.partition_broadcast(r_bc[:], r_sb[:])
                        nc.vector.tensor_mul(att[h][:], attu[:], r_bc[:])
                    # previous q block's output projection, emitted here so
                    # its PE work queues behind this block's attention and
                    # never stalls the in-order PE stream on normalization
                    if pending is not None:
                        emit_wo(*pending)
                    pending = (att, q0)
                if pending is not None:
                    emit_wo(*pending)
    nc.compile()
    return nc


_program = None


def _get_program():
    global _program
    if _program is None:
        _program = _build_program()
    return _program


def kernel(**inputs) -> np.ndarray:
    x = np.asarray(inputs["x"], dtype=np.float32)
    wq = np.asarray(inputs["wq"], dtype=np.float32)
    wk = np.asarray(inputs["wk"], dtype=np.float32)
    wv = np.asarray(inputs["wv"], dtype=np.float32)
    wo = np.asarray(inputs["wo"], dtype=np.float32)
    cos = np.asarray(inputs["freqs_cos"], dtype=np.float32)
    sin = np.asarray(inputs["freqs_sin"], dtype=np.float32)
    mask = np.asarray(inputs["mask"], dtype=np.float32)
    start_pos = int(np.asarray(inputs.get("start_pos", 0)))
    assert start_pos == 0, "kernel specialized for start_pos == 0"

    # Even/odd RoPE pair split within each head's 128 features.
    perm = np.concatenate([np.arange(0, HD, 2), np.arange(1, HD, 2)])

    xT = np.ascontiguousarray(x.reshape(T, D).T)
    cosT = cos.T                                   # [64, S]
    sinT = sin.T
    ropc = np.ascontiguousarray(np.concatenate([cosT, cosT], axis=0))
    rops = np.ascontiguousarray(np.concatenate([-sinT, sinT], axis=0))
    # Transposed diagonal-mask variants: for kt chunk at offset v*128 within
    # a 512-token q block, maskt_v[r, c] = clamp(mask[c, v*128 + r]).
    masktv = [np.ascontiguousarray(
        np.maximum(mask[:512, v * 128:(v + 1) * 128], NEG).astype(np.float32).T)
        for v in range(4)]

    in_maps = []
    for c in range(N_CORES):
        wq_c = wq[c * FL:(c + 1) * FL].reshape(QH, HD, D)[:, perm, :].reshape(FL, D)
        wk_c = wk[c * HD:(c + 1) * HD][perm, :]
        wv_c = wv[c * HD:(c + 1) * HD]
        wo_c = wo[:, c * FL:(c + 1) * FL]
        in_maps.append({
            "xT": xT,
            "idin": np.eye(128, dtype=np.float32),
            "wqT": np.ascontiguousarray(wq_c.T),
            "wkT": np.ascontiguousarray(wk_c.T),
            "wvT": np.ascontiguousarray(wv_c.T),
            "woT": np.ascontiguousarray(wo_c.T),
            "ropc": ropc,
            "rops": rops,
            "onesin": np.ones((128, 1), dtype=np.float32),
            "maskt0": masktv[0],
            "maskt1": masktv[1],
            "maskt2": masktv[2],
            "maskt3": masktv[3],
        })

    nc = _get_program()
    trace = bool(int(os.environ.get("GQA_TRACE", "0")))
    kwargs = {}
    if trace:
        tmpdir = os.environ.get("GQA_TRACE_DIR") or None
        kwargs = dict(trace=True, tmpdir=tmpdir, trace_cores=[0])
    res = run_bass_kernel_spmd(nc, in_maps, list(range(N_CORES)), **kwargs)
    kernel.last_results = res

    acc = np.zeros((T, D), dtype=np.float64)
    for c in range(N_CORES):
        acc += res.results[c]["y"]
    return acc.astype(np.float32).reshape(B, S, D)



# revision 2
# speedup vs baseline: 1.1558x; 1.1558x over previous
"""GQA attention layer (B=2, S=2048, D=4096, 32 Q heads / 8 KV heads, RoPE,
causal) on 8 Trainium2 NeuronCores, tensor-parallel over heads.

Each core owns 4 Q heads + 1 KV head and computes the whole layer for its
slice in ONE fused pass: per 1024-token half-batch it projects Q/K/V
(bf16 operands, fp32 PSUM accumulation), applies RoPE straight out of
PSUM on the vector engine (swap-free half-partition formulation), runs
causal attention out of SBUF-resident K/V, and emits a streamed output
projection deferred by one half so normalization latency never stalls
the PE.  The host sums the 8 partial outputs (bf16 on the wire).

Key layout choices:
 - activations feature-major [feature_partition, token_free]; every
   matmul contracts over the partition dim.
 - projection PSUM packs two 256-wide outputs per 2KB bank using the
   per-element has_written semantics (single start=True per bank).
 - V is transposed to token-major on the PE (deferred batch of
   transposes per half, packed 2 per PSUM bank).
 - softmax denominators: exp tiles accumulated on DVE, reduced across
   partitions + broadcast in one gpsimd partition_all_reduce.
"""

import os
import sys
import types
from contextlib import ExitStack

import numpy as np
import ml_dtypes

import concourse.bass as bass
import concourse.tile as tile
from concourse import bacc
from concourse import mybir
from concourse import bass_isa
from concourse import bass_utils
from concourse.bass_utils import run_bass_kernel_spmd

# Optional NTFF profiling support under axon (trimmed image lacks
# antenv.axon_hooks); harmless when unavailable.
try:
    import antenv  # noqa: F401
    from trn_agent_boot.trn_boot import _ntff_profile_via_ctypes

    if "antenv.axon_hooks" not in sys.modules:
        _hooks_mod = types.ModuleType("antenv.axon_hooks")
        _hook = _ntff_profile_via_ctypes("/opt/axon/libaxon_pjrt.so")
        _hooks_mod.get_axon_ntff_profile_hook = lambda: _hook
        _hooks_mod.set_axon_ntff_profile_hook = lambda h: None
        sys.modules["antenv.axon_hooks"] = _hooks_mod
    bass_utils.upload_artifacts = lambda tmpdir: "local://skipped"
except Exception:
    pass

F32 = mybir.dt.float32
F32R = mybir.dt.float32r
BF16 = mybir.dt.bfloat16
EXP = mybir.ActivationFunctionType.Exp

B, S, D = 2, 2048, 4096
NH, NKV, HD = 32, 8, 128
T = B * S
N_CORES = 8
QH = NH // N_CORES              # 4 local q heads
FL = QH * HD                    # 512 local q features
SCALE = 1.0 / float(np.sqrt(HD))
NEG = -1.0e30

NW = 256                        # tokens per projection group
HALF = 1024                     # tokens per fused pipeline stage
NG = HALF // NW                 # 4 proj groups per half
QB = 512                        # q-block width in attention
DKD = D // 128                  # 32 contraction chunks
NJB = 4                         # x/wq chunk-groups (8 k-chunks each)
NDG = D // QB                   # 8 output-projection column blocks


def _build_program():
    nc = bacc.Bacc("TRN2", target_bir_lowering=False, debug=False,
                   num_devices=N_CORES)

    # Host-pretiled inputs (see kernel() for exact layouts).
    x4 = nc.dram_tensor("x4", [16 * NJB * 128, 8 * NW], BF16,
                        kind="ExternalInput").ap()
    wq4 = nc.dram_tensor("wq4", [128, DKD * FL], BF16, kind="ExternalInput").ap()
    wk4 = nc.dram_tensor("wk4", [128, DKD * HD], BF16, kind="ExternalInput").ap()
    wv4 = nc.dram_tensor("wv4", [128, DKD * HD], BF16, kind="ExternalInput").ap()
    wot = nc.dram_tensor("wot", [NDG * 128, QH * QB], F32R,
                         kind="ExternalInput").ap()
    ropc = nc.dram_tensor("ropc", [HD, S], F32, kind="ExternalInput").ap()
    rops = nc.dram_tensor("rops", [HD, S], F32, kind="ExternalInput").ap()
    idin = nc.dram_tensor("idin", [128, 128], F32R, kind="ExternalInput").ap()
    maskt = [nc.dram_tensor(f"maskt{v}", [128, QB], F32,
                            kind="ExternalInput").ap() for v in range(4)]
    # Output: y_t[tg, dg] = y[tg*128:(tg+1)*128, dg*512:(dg+1)*512] in bf16.
    y_t = nc.dram_tensor("y_t", [(T // 128) * NDG * 128, QB], BF16,
                         kind="ExternalOutput").ap()

    with tile.TileContext(nc) as tc, ExitStack() as ctx:
        # ------------------------------------------------------------------
        # Persistent SBUF tiles
        # ------------------------------------------------------------------
        const = ctx.enter_context(tc.tile_pool(name="const", bufs=1))
        ident = const.tile([128, 128], F32R)
        nc.sync.dma_start(ident[:], idin)
        mtv = []
        for v in range(4):
            mt = const.tile([128, QB], F32, tag=f"mtv{v}", name=f"mtv{v}")
            nc.sync.dma_start(mt[:], maskt[v])
            mtv.append(mt)

        wpool = ctx.enter_context(tc.tile_pool(name="weights", bufs=1))
        wq_sb = wpool.tile([128, DKD * FL], BF16, tag="wq")
        for jb in range(NJB):
            nc.sync.dma_start(
                wq_sb[:, jb * 8 * FL:(jb + 1) * 8 * FL],
                wq4[:, jb * 8 * FL:(jb + 1) * 8 * FL])
        wk_sb = wpool.tile([128, DKD * HD], BF16, tag="wk")
        nc.sync.dma_start(wk_sb[:], wk4)
        wv_sb = wpool.tile([128, DKD * HD], BF16, tag="wv")
        nc.sync.dma_start(wv_sb[:], wv4)

        rcpool = ctx.enter_context(tc.tile_pool(name="ropec", bufs=1))
        cos_s = rcpool.tile([HD, HALF], F32, tag="cos")
        sin_s = rcpool.tile([HD, HALF], F32, tag="sin")

        apool = ctx.enter_context(tc.tile_pool(name="acts", bufs=1))
        q_half = [apool.tile([128, HALF], F32R, tag=f"qh{m}", name=f"qh{m}")
                  for m in range(QH)]
        kT = apool.tile([128, S], F32R, tag="kT")
        V_b = apool.tile([128, S], F32R, tag="V_b")

        attpool = ctx.enter_context(tc.tile_pool(name="att", bufs=2))
        wopool = ctx.enter_context(tc.tile_pool(name="wostream", bufs=2))
        xpool = ctx.enter_context(tc.tile_pool(name="xin", bufs=3))
        ptpool = ctx.enter_context(tc.tile_pool(name="pt", bufs=3))
        accpool = ctx.enter_context(tc.tile_pool(name="acc", bufs=2))
        smpool = ctx.enter_context(tc.tile_pool(name="sums", bufs=2))
        aupool = ctx.enter_context(tc.tile_pool(name="attun", bufs=2))
        yspool = ctx.enter_context(tc.tile_pool(name="ystage", bufs=4))
        vstpool = ctx.enter_context(tc.tile_pool(name="vst", bufs=4))
        rtpool = ctx.enter_context(tc.tile_pool(name="ropetmp", bufs=2))
        ypsum = ctx.enter_context(tc.tile_pool(name="yps", bufs=2, space="PSUM"))

        def rope_evict(ps, out, g):
            """RoPE on the even/odd-split feature layout, swap-free:
            out[0:64]  = ps[0:64]*cos + ps[64:128]*(-sin)
            out[64:128]= ps[64:128]*cos + ps[0:64]*(+sin)
            (cos_s rows duplicated; sin_s rows = [-sin; sin]).
            Reads ps (PSUM) directly on the vector engine."""
            c0 = g * NW
            tmp = rtpool.tile([128, NW], F32, tag="rt")
            nc.vector.tensor_mul(tmp[0:64, :], ps[64:128, :],
                                 sin_s[0:64, c0:c0 + NW])
            nc.vector.tensor_mul(out[0:64, :], ps[0:64, :],
                                 cos_s[0:64, c0:c0 + NW])
            nc.vector.tensor_add(out[0:64, :], out[0:64, :], tmp[0:64, :])
            nc.vector.tensor_mul(tmp[64:128, :], ps[0:64, :],
                                 sin_s[64:128, c0:c0 + NW])
            nc.vector.tensor_mul(out[64:128, :], ps[64:128, :],
                                 cos_s[64:128, c0:c0 + NW])
            nc.vector.tensor_add(out[64:128, :], out[64:128, :],
                                 tmp[64:128, :])

        def emit_wo(att_h, b, hb):
            """Output projection for one 1024-token half: y-slice =
            att_h (4x[128,1024] feature-major) contracted with streamed
            wo column blocks."""
            tg0 = b * (S // 128) + hb * (HALF // 128)
            for dg in range(NDG):
                wod = wopool.tile([128, QH * QB], F32R, tag="wod")
                nc.sync.dma_start(wod[:], wot[dg * 128:(dg + 1) * 128, :])
                for tcx in range(HALF // 128):
                    yp = ypsum.tile([128, QB], F32, tag="yp")
                    for f in range(QH):
                        nc.tensor.matmul(
                            yp[:], att_h[f][:, tcx * 128:(tcx + 1) * 128],
                            wod[:, f * QB:(f + 1) * QB],
                            start=(f == 0), stop=(f == QH - 1))
                    ysb = yspool.tile([128, QB], BF16, tag="ysb")
                    nc.scalar.copy(ysb[:], yp[:])
                    tg = tg0 + tcx
                    nc.sync.dma_start(
                        y_t[(tg * NDG + dg) * 128:(tg * NDG + dg + 1) * 128, :],
                        ysb[:])

        pending = None
        for b in range(B):
            for hb in range(2):
                base = hb * HALF            # position within batch
                # rope constants for this half's positions
                nc.sync.dma_start(cos_s[:], ropc[:, base:base + HALF])
                nc.sync.dma_start(sin_s[:], rops[:, base:base + HALF])

                # ----------------------------------------------------------
                # QKV projection + RoPE for this half
                # ----------------------------------------------------------
                with tc.tile_pool(name="projps", bufs=1, space="PSUM") as pps:
                    vsts = []
                    for g in range(NG):
                        gg = (b * S + base) // NW + g   # global 256-tok group
                        qab = pps.tile([128, 2 * NW], F32, tag="qab", bufs=2)
                        qcd = pps.tile([128, 2 * NW], F32, tag="qcd", bufs=2)
                        kv = pps.tile([128, 2 * NW], F32, tag="kv", bufs=1)
                        for jb in range(NJB):
                            xg = xpool.tile([128, 8 * NW], BF16, tag="xg")
                            r0 = (gg * NJB + jb) * 128
                            nc.sync.dma_start(xg[:], x4[r0:r0 + 128, :])
                            for c in range(8):
                                k = jb * 8 + c
                                xs = xg[:, c * NW:(c + 1) * NW]
                                st = (k == 0)
                                sp = (k == DKD - 1)
                                # kv bank first: unblocks next group earliest
                                nc.tensor.matmul(
                                    kv[:, 0:NW],
                                    wk_sb[:, k * HD:(k + 1) * HD], xs,
                                    start=st, stop=False)
                                nc.tensor.matmul(
                                    kv[:, NW:2 * NW],
                                    wv_sb[:, k * HD:(k + 1) * HD], xs,
                                    start=False, stop=sp)
                                nc.tensor.matmul(
                                    qab[:, 0:NW],
                                    wq_sb[:, k * FL:k * FL + 128], xs,
                                    start=st, stop=False)
                                nc.tensor.matmul(
                                    qab[:, NW:2 * NW],
                                    wq_sb[:, k * FL + 128:k * FL + 256], xs,
                                    start=False, stop=sp)
                                nc.tensor.matmul(
                                    qcd[:, 0:NW],
                                    wq_sb[:, k * FL + 256:k * FL + 384], xs,
                                    start=st, stop=False)
                                nc.tensor.matmul(
                                    qcd[:, NW:2 * NW],
                                    wq_sb[:, k * FL + 384:k * FL + 512], xs,
                                    start=False, stop=sp)
                        # evictions: k rope + v copy first (kv is bufs=1)
                        rope_evict(kv[:, 0:NW],
                                   kT[:, base + g * NW:base + (g + 1) * NW], g)
                        vst = vstpool.tile([128, NW], F32R, tag="vst")
                        nc.scalar.copy(vst[:], kv[:, NW:2 * NW])
                        vsts.append(vst)
                        rope_evict(qab[:, 0:NW],
                                   q_half[0][:, g * NW:(g + 1) * NW], g)
                        rope_evict(qab[:, NW:2 * NW],
                                   q_half[1][:, g * NW:(g + 1) * NW], g)
                        rope_evict(qcd[:, 0:NW],
                                   q_half[2][:, g * NW:(g + 1) * NW], g)
                        rope_evict(qcd[:, NW:2 * NW],
                                   q_half[3][:, g * NW:(g + 1) * NW], g)
                    # deferred V transposes (token-major V_b), 2 per bank
                    for g in range(NG):
                        vtp = pps.tile([128, NW], F32R, tag="vtp", bufs=1)
                        nc.tensor.transpose(
                            vtp[:, 0:128], vsts[g][:, 0:128], ident[:])
                        nc.tensor.transpose(
                            vtp[:, 128:256], vsts[g][:, 128:256], ident[:])
                        nc.vector.tensor_copy(
                            V_b[:, base + g * NW:base + (g + 1) * NW], vtp[:])

                # ----------------------------------------------------------
                # Attention for this half (q blocks of 512)
                # ----------------------------------------------------------
                att_h = [attpool.tile([128, HALF], F32R, tag=f"at{f}",
                                      name=f"at{f}") for f in range(QH)]
                with tc.tile_pool(name="attnps", bufs=1, space="PSUM") as aps:
                    for qb in range(HALF // QB):
                        q0 = qb * QB
                        nkt = (base + q0 + QB) // 128
                        for h in range(QH):
                            avp = aps.tile([128, QB], F32, tag="avp", bufs=2)
                            acc = accpool.tile([128, QB], F32R, tag="acc")
                            for ktc in range(nkt):
                                stp = aps.tile([128, QB], F32, tag="stp",
                                               bufs=2)
                                nc.tensor.matmul(
                                    stp[:], kT[:, ktc * 128:(ktc + 1) * 128],
                                    q_half[h][:, q0:q0 + QB],
                                    start=True, stop=True)
                                if ktc >= nkt - 4:
                                    nc.vector.tensor_add(
                                        stp[:], stp[:], mtv[ktc - (nkt - 4)][:])
                                pt = ptpool.tile([128, QB], F32R, tag="pt")
                                nc.scalar.activation(pt[:], stp[:], EXP,
                                                     scale=SCALE)
                                nc.tensor.matmul(
                                    avp[:], V_b[:, ktc * 128:(ktc + 1) * 128],
                                    pt[:], start=(ktc == 0),
                                    stop=(ktc == nkt - 1))
                                if ktc == 0:
                                    nc.vector.tensor_copy(acc[:], pt[:])
                                else:
                                    nc.vector.tensor_add(acc[:], acc[:], pt[:])
                            att_un = aupool.tile([128, QB], F32R, tag="au")
                            nc.scalar.copy(att_un[:], avp[:])
                            sums = smpool.tile([128, QB], F32, tag="sm")
                            nc.gpsimd.partition_all_reduce(
                                sums[:], acc[:], 128, bass_isa.ReduceOp.add)
                            nc.vector.reciprocal(sums[:], sums[:])
                            nc.vector.tensor_mul(
                                att_h[h][:, q0:q0 + QB], att_un[:], sums[:])
                # previous half's output projection, emitted here so its PE
                # work queues behind this half's attention and the
                # normalization chain never stalls the in-order PE stream
                if pending is not None:
                    emit_wo(*pending)
                pending = (att_h, b, hb)
        emit_wo(*pending)
    nc.compile()
    return nc


_program = None


def _get_program():
    global _program
    if _program is None:
        _program = _build_program()
    return _program


def kernel(**inputs) -> np.ndarray:
    x = np.asarray(inputs["x"], dtype=np.float32)
    wq = np.asarray(inputs["wq"], dtype=np.float32)
    wk = np.asarray(inputs["wk"], dtype=np.float32)
    wv = np.asarray(inputs["wv"], dtype=np.float32)
    wo = np.asarray(inputs["wo"], dtype=np.float32)
    cos = np.asarray(inputs["freqs_cos"], dtype=np.float32)
    sin = np.asarray(inputs["freqs_sin"], dtype=np.float32)
    mask = np.asarray(inputs["mask"], dtype=np.float32)
    start_pos = int(np.asarray(inputs.get("start_pos", 0)))
    assert start_pos == 0, "kernel specialized for start_pos == 0"

    # Even/odd RoPE pair split within each head's 128 features.
    perm = np.concatenate([np.arange(0, HD, 2), np.arange(1, HD, 2)])

    # x tiled: x4[gg, jb] rows = [128, 8*256] where row p, col c*256+w =
    # x_token[gg*256 + w, (jb*8+c)*128 + p]
    xT = x.reshape(T, D).T                              # [D, T]
    x4 = np.ascontiguousarray(
        xT.reshape(NJB, 8, 128, 16, NW).transpose(3, 0, 2, 1, 4)
        .reshape(16 * NJB * 128, 8 * NW)).astype(ml_dtypes.bfloat16)

    cosT = cos.T                                        # [64, S]
    sinT = sin.T
    ropc = np.ascontiguousarray(np.concatenate([cosT, cosT], axis=0))
    rops = np.ascontiguousarray(np.concatenate([-sinT, sinT], axis=0))
    masktv = [np.ascontiguousarray(
        np.maximum(mask[:QB, v * 128:(v + 1) * 128], NEG)
        .astype(np.float32).T) for v in range(4)]

    in_maps = []
    for c in range(N_CORES):
        wq_c = (wq[c * FL:(c + 1) * FL].reshape(QH, HD, D)[:, perm, :]
                .reshape(FL, D))
        wk_c = wk[c * HD:(c + 1) * HD][perm, :]
        wv_c = wv[c * HD:(c + 1) * HD]
        wo_c = wo[:, c * FL:(c + 1) * FL]
        # wq4[p, k*512 + f] = wq_c[f, k*128+p]  (k-chunk-major, bf16)
        wq4 = np.ascontiguousarray(
            wq_c.T.reshape(DKD, 128, FL).transpose(1, 0, 2)
            .reshape(128, DKD * FL)).astype(ml_dtypes.bfloat16)
        wk4 = np.ascontiguousarray(
            wk_c.T.reshape(DKD, 128, HD).transpose(1, 0, 2)
            .reshape(128, DKD * HD)).astype(ml_dtypes.bfloat16)
        wv4 = np.ascontiguousarray(
            wv_c.T.reshape(DKD, 128, HD).transpose(1, 0, 2)
            .reshape(128, DKD * HD)).astype(ml_dtypes.bfloat16)
        # wot[dg*128+p, f*512+c] = wo_c[dg*512+c, f*128+p]
        wot = np.ascontiguousarray(
            wo_c.T.reshape(QH, 128, NDG, QB).transpose(2, 1, 0, 3)
            .reshape(NDG * 128, QH * QB))
        in_maps.append({
            "x4": x4,
            "wq4": wq4,
            "wk4": wk4,
            "wv4": wv4,
            "wot": wot,
            "ropc": ropc,
            "rops": rops,
            "idin": np.eye(128, dtype=np.float32),
            "maskt0": masktv[0],
            "maskt1": masktv[1],
            "maskt2": masktv[2],
            "maskt3": masktv[3],
        })

    nc = _get_program()
    trace = bool(int(os.environ.get("GQA_TRACE", "0")))
    kwargs = {}
    if trace:
        tmpdir = os.environ.get("GQA_TRACE_DIR") or None
        kwargs = dict(trace=True, tmpdir=tmpdir, trace_cores=[0])
    res = run_bass_kernel_spmd(nc, in_maps, list(range(N_CORES)), **kwargs)
    kernel.last_results = res

    acc = np.zeros((T // 128, 128, D), dtype=np.float64)
    for c in range(N_CORES):
        yt = np.asarray(res.results[c]["y_t"], dtype=np.float64)
        acc += yt.reshape(T // 128, NDG, 128, QB).transpose(0, 2, 1, 3) \
                 .reshape(T // 128, 128, D)
    return acc.astype(np.float32).reshape(B, S, D)


# revision 9
# speedup vs baseline: 1.1975x; 1.0360x over previous
"""GQA attention layer (B=2, S=2048, D=4096, 32 Q heads / 8 KV heads, RoPE,
causal) on 8 Trainium2 NeuronCores, tensor-parallel over heads.

Each core owns 4 Q heads + 1 KV head and computes the whole layer for its
slice in ONE fused pass: per 1024-token half-batch it projects Q/K/V
(bf16 operands, fp32 PSUM accumulation), applies RoPE straight out of
PSUM on the vector engine (swap-free half-partition formulation), runs
causal attention out of SBUF-resident K/V, and emits a streamed output
projection deferred by one half so normalization latency never stalls
the PE.  The host sums the 8 partial outputs (bf16 on the wire).

Key layout choices:
 - activations feature-major [feature_partition, token_free]; every
   matmul contracts over the partition dim.
 - projection PSUM packs two 256-wide outputs per 2KB bank using the
   per-element has_written semantics (single start=True per bank).
 - V is transposed to token-major on the PE (deferred batch of
   transposes per half, packed 2 per PSUM bank).
 - softmax denominators: exp tiles accumulated on DVE, reduced across
   partitions + broadcast in one gpsimd partition_all_reduce.
"""

import os
import sys
import types
from contextlib import ExitStack

import numpy as np
import ml_dtypes

import concourse.bass as bass
import concourse.tile as tile
from concourse import bacc
from concourse import mybir
from concourse import bass_utils
from concourse.bass_utils import run_bass_kernel_spmd

# Optional NTFF profiling support under axon (trimmed image lacks
# antenv.axon_hooks); harmless when unavailable.
try:
    import antenv  # noqa: F401
    from trn_agent_boot.trn_boot import _ntff_profile_via_ctypes

    if "antenv.axon_hooks" not in sys.modules:
        _hooks_mod = types.ModuleType("antenv.axon_hooks")
        _hook = _ntff_profile_via_ctypes("/opt/axon/libaxon_pjrt.so")
        _hooks_mod.get_axon_ntff_profile_hook = lambda: _hook
        _hooks_mod.set_axon_ntff_profile_hook = lambda h: None
        sys.modules["antenv.axon_hooks"] = _hooks_mod
    bass_utils.upload_artifacts = lambda tmpdir: "local://skipped"
except Exception:
    pass

F32 = mybir.dt.float32
F32R = mybir.dt.float32r
BF16 = mybir.dt.bfloat16
EXP = mybir.ActivationFunctionType.Exp

B, S, D = 2, 2048, 4096
NH, NKV, HD = 32, 8, 128
T = B * S
N_CORES = 8
QH = NH // N_CORES              # 4 local q heads
FL = QH * HD                    # 512 local q features
SCALE = 1.0 / float(np.sqrt(HD))
NEG = -1.0e30

NW = 256                        # tokens per projection group
HALF = 1024                     # tokens per fused pipeline stage
NG = HALF // NW                 # 4 proj groups per half
QB = 512                        # q-block width in attention
DKD = D // 128                  # 32 contraction chunks
NJB = 4                         # x/wq chunk-groups (8 k-chunks each)
NDG = D // QB                   # 8 output-projection column blocks


def _build_program():
    nc = bacc.Bacc("TRN2", target_bir_lowering=False, debug=False,
                   num_devices=N_CORES)

    # Host-pretiled inputs (see kernel() for exact layouts).
    x4 = nc.dram_tensor("x4", [16 * NJB * 128, 8 * NW], BF16,
                        kind="ExternalInput").ap()
    wq4 = nc.dram_tensor("wq4", [128, DKD * FL], BF16, kind="ExternalInput").ap()
    wk4 = nc.dram_tensor("wk4", [128, DKD * HD], BF16, kind="ExternalInput").ap()
    wv4 = nc.dram_tensor("wv4", [128, DKD * HD], BF16, kind="ExternalInput").ap()
    wot = nc.dram_tensor("wot", [NDG * 128, QH * QB], F32R,
                         kind="ExternalInput").ap()
    ropc = nc.dram_tensor("ropc", [HD, S], F32, kind="ExternalInput").ap()
    rops = nc.dram_tensor("rops", [HD, S], F32, kind="ExternalInput").ap()
    idin = nc.dram_tensor("idin", [128, 128], F32R, kind="ExternalInput").ap()
    onesin = nc.dram_tensor("onesin", [128, 1], F32R, kind="ExternalInput").ap()
    maskt = [nc.dram_tensor(f"maskt{v}", [128, QB], F32,
                            kind="ExternalInput").ap() for v in range(4)]
    # Output: y_t[tg, dg] = y[tg*128:(tg+1)*128, dg*512:(dg+1)*512] in bf16.
    y_t = nc.dram_tensor("y_t", [(T // 128) * NDG * 128, QB], BF16,
                         kind="ExternalOutput").ap()

    with tile.TileContext(nc) as tc, ExitStack() as ctx:
        # ------------------------------------------------------------------
        # Persistent SBUF tiles
        # ------------------------------------------------------------------
        # Weight loads first (wq chunk-group 0 before everything else so the
        # first projection matmuls can start within a few microseconds).
        wpool = ctx.enter_context(tc.tile_pool(name="weights", bufs=1))
        wq_sb = wpool.tile([128, DKD * FL], BF16, tag="wq")
        nc.sync.dma_start(wq_sb[:, 0:8 * FL], wq4[:, 0:8 * FL])
        wk_sb = wpool.tile([128, DKD * HD], BF16, tag="wk")
        nc.sync.dma_start(wk_sb[:], wk4)
        wv_sb = wpool.tile([128, DKD * HD], BF16, tag="wv")
        nc.sync.dma_start(wv_sb[:], wv4)
        for jb in range(1, NJB):
            nc.sync.dma_start(
                wq_sb[:, jb * 8 * FL:(jb + 1) * 8 * FL],
                wq4[:, jb * 8 * FL:(jb + 1) * 8 * FL])

        const = ctx.enter_context(tc.tile_pool(name="const", bufs=1))
        ident = const.tile([128, 128], F32R)
        nc.sync.dma_start(ident[:], idin)
        ones_t = const.tile([128, 1], F32R, tag="ones")
        nc.sync.dma_start(ones_t[:], onesin)
        mtv = []
        for v in range(4):
            mt = const.tile([128, QB], F32, tag=f"mtv{v}", name=f"mtv{v}")
            nc.sync.dma_start(mt[:], maskt[v])
            mtv.append(mt)

        rcpool = ctx.enter_context(tc.tile_pool(name="ropec", bufs=1))
        cos_s = rcpool.tile([HD, HALF], F32, tag="cos")
        sin_s = rcpool.tile([HD, HALF], F32, tag="sin")

        apool = ctx.enter_context(tc.tile_pool(name="acts", bufs=1))
        q_half = [apool.tile([128, HALF], F32R, tag=f"qh{m}", name=f"qh{m}")
                  for m in range(QH)]
        kT = apool.tile([128, S], F32R, tag="kT")
        V_b = apool.tile([128, S], F32R, tag="V_b")

        attpool = ctx.enter_context(tc.tile_pool(name="att", bufs=2))
        wopool = ctx.enter_context(tc.tile_pool(name="wostream", bufs=2))
        xpool = ctx.enter_context(tc.tile_pool(name="xin", bufs=3))
        ptpool = ctx.enter_context(tc.tile_pool(name="pt", bufs=3))
        accpool = ctx.enter_context(tc.tile_pool(name="acc", bufs=2))
        smpool = ctx.enter_context(tc.tile_pool(name="sums", bufs=2))
        aupool = ctx.enter_context(tc.tile_pool(name="attun", bufs=2))
        yspool = ctx.enter_context(tc.tile_pool(name="ystage", bufs=4))
        vstpool = ctx.enter_context(tc.tile_pool(name="vst", bufs=4))
        rtpool = ctx.enter_context(tc.tile_pool(name="ropetmp", bufs=2))
        ypsum = ctx.enter_context(tc.tile_pool(name="yps", bufs=2, space="PSUM"))

        def rope_evict(ps, out, g):
            """RoPE on the even/odd-split feature layout, swap-free:
            out[0:64]  = ps[0:64]*cos + ps[64:128]*(-sin)
            out[64:128]= ps[64:128]*cos + ps[0:64]*(+sin)
            (cos_s rows duplicated; sin_s rows = [-sin; sin]).
            Reads ps (PSUM) directly on the vector engine."""
            c0 = g * NW
            tmp = rtpool.tile([128, NW], F32, tag="rt")
            nc.vector.tensor_mul(tmp[0:64, :], ps[64:128, :],
                                 sin_s[0:64, c0:c0 + NW])
            nc.vector.tensor_mul(out[0:64, :], ps[0:64, :],
                                 cos_s[0:64, c0:c0 + NW])
            nc.vector.tensor_add(out[0:64, :], out[0:64, :], tmp[0:64, :])
            nc.vector.tensor_mul(tmp[64:128, :], ps[0:64, :],
                                 sin_s[64:128, c0:c0 + NW])
            nc.vector.tensor_mul(out[64:128, :], ps[64:128, :],
                                 cos_s[64:128, c0:c0 + NW])
            nc.vector.tensor_add(out[64:128, :], out[64:128, :],
                                 tmp[64:128, :])

        def emit_wo(att_h, b, hb):
            """Output projection for one 1024-token half: y-slice =
            att_h (4x[128,1024] feature-major) contracted with streamed
            wo column blocks."""
            tg0 = b * (S // 128) + hb * (HALF // 128)
            for dg in range(NDG):
                wod = wopool.tile([128, QH * QB], F32R, tag="wod")
                nc.sync.dma_start(wod[:], wot[dg * 128:(dg + 1) * 128, :])
                for tcx in range(HALF // 128):
                    yp = ypsum.tile([128, QB], F32, tag="yp")
                    for f in range(QH):
                        nc.tensor.matmul(
                            yp[:], att_h[f][:, tcx * 128:(tcx + 1) * 128],
                            wod[:, f * QB:(f + 1) * QB],
                            start=(f == 0), stop=(f == QH - 1))
                    ysb = yspool.tile([128, QB], BF16, tag="ysb")
                    nc.vector.tensor_copy(ysb[:], yp[:])
                    tg = tg0 + tcx
                    nc.sync.dma_start(
                        y_t[(tg * NDG + dg) * 128:(tg * NDG + dg + 1) * 128, :],
                        ysb[:])

        pending = None
        for b in range(B):
            for hb in range(2):
                base = hb * HALF            # position within batch
                # rope constants for this half's positions
                nc.sync.dma_start(cos_s[:], ropc[:, base:base + HALF])
                nc.sync.dma_start(sin_s[:], rops[:, base:base + HALF])

                # ----------------------------------------------------------
                # QKV projection + RoPE for this half
                # ----------------------------------------------------------
                with tc.tile_pool(name="projps", bufs=1, space="PSUM") as pps:
                    vsts = []
                    for g in range(NG):
                        gg = (b * S + base) // NW + g   # global 256-tok group
                        qab = pps.tile([128, 2 * NW], F32, tag="qab", bufs=2)
                        qcd = pps.tile([128, 2 * NW], F32, tag="qcd", bufs=2)
                        kv = pps.tile([128, 2 * NW], F32, tag="kv", bufs=1)
                        for jb in range(NJB):
                            xg = xpool.tile([128, 8 * NW], BF16, tag="xg")
                            r0 = (gg * NJB + jb) * 128
                            nc.sync.dma_start(xg[:], x4[r0:r0 + 128, :])
                            # q matmuls first, kv last: the kv bank is
                            # single-buffered, so its previous-group RoPE /
                            # copy reads get ~3.4us of q-matmul cover before
                            # the next write touches the bank.
                            for c in range(8):
                                k = jb * 8 + c
                                xs = xg[:, c * NW:(c + 1) * NW]
                                st = (k == 0)
                                sp = (k == DKD - 1)
                                nc.tensor.matmul(
                                    qab[:, 0:NW],
                                    wq_sb[:, k * FL:k * FL + 128], xs,
                                    start=st, stop=False)
                                nc.tensor.matmul(
                                    qab[:, NW:2 * NW],
                                    wq_sb[:, k * FL + 128:k * FL + 256], xs,
                                    start=False, stop=sp)
                                nc.tensor.matmul(
                                    qcd[:, 0:NW],
                                    wq_sb[:, k * FL + 256:k * FL + 384], xs,
                                    start=st, stop=False)
                                nc.tensor.matmul(
                                    qcd[:, NW:2 * NW],
                                    wq_sb[:, k * FL + 384:k * FL + 512], xs,
                                    start=False, stop=sp)
                            for c in range(8):
                                k = jb * 8 + c
                                xs = xg[:, c * NW:(c + 1) * NW]
                                st = (k == 0)
                                sp = (k == DKD - 1)
                                nc.tensor.matmul(
                                    kv[:, 0:NW],
                                    wk_sb[:, k * HD:(k + 1) * HD], xs,
                                    start=st, stop=False)
                                nc.tensor.matmul(
                                    kv[:, NW:2 * NW],
                                    wv_sb[:, k * HD:(k + 1) * HD], xs,
                                    start=False, stop=sp)
                        # evictions: k rope + v copy first (kv is bufs=1)
                        rope_evict(kv[:, 0:NW],
                                   kT[:, base + g * NW:base + (g + 1) * NW], g)
                        vst = vstpool.tile([128, NW], F32R, tag="vst")
                        nc.scalar.copy(vst[:], kv[:, NW:2 * NW])
                        vsts.append(vst)
                        rope_evict(qab[:, 0:NW],
                                   q_half[0][:, g * NW:(g + 1) * NW], g)
                        rope_evict(qab[:, NW:2 * NW],
                                   q_half[1][:, g * NW:(g + 1) * NW], g)
                        rope_evict(qcd[:, 0:NW],
                                   q_half[2][:, g * NW:(g + 1) * NW], g)
                        rope_evict(qcd[:, NW:2 * NW],
                                   q_half[3][:, g * NW:(g + 1) * NW], g)
                    # deferred V transposes (token-major V_b), 2 per bank
                    for g in range(NG):
                        vtp = pps.tile([128, NW], F32R, tag="vtp", bufs=1)
                        nc.tensor.transpose(
                            vtp[:, 0:128], vsts[g][:, 0:128], ident[:])
                        nc.tensor.transpose(
                            vtp[:, 128:256], vsts[g][:, 128:256], ident[:])
                        nc.vector.tensor_copy(
                            V_b[:, base + g * NW:base + (g + 1) * NW], vtp[:])

                # ----------------------------------------------------------
                # Attention for this half (q blocks of 512)
                # ----------------------------------------------------------
                att_h = [attpool.tile([128, HALF], F32R, tag=f"at{f}",
                                      name=f"at{f}") for f in range(QH)]
                with tc.tile_pool(name="attnps", bufs=1, space="PSUM") as aps:
                    for qb in range(HALF // QB):
                        q0 = qb * QB
                        nkt = (base + q0 + QB) // 128
                        for h in range(QH):
                            avp = aps.tile([128, QB], F32, tag="avp", bufs=2)
                            smp = aps.tile([1, QB], F32, tag="smp", bufs=2)
                            for ktc in range(nkt):
                                stp = aps.tile([128, QB], F32, tag="stp",
                                               bufs=2)
                                nc.tensor.matmul(
                                    stp[:], kT[:, ktc * 128:(ktc + 1) * 128],
                                    q_half[h][:, q0:q0 + QB],
                                    start=True, stop=True)
                                if ktc >= nkt - 4:
                                    nc.vector.tensor_add(
                                        stp[:], stp[:], mtv[ktc - (nkt - 4)][:])
                                pt = ptpool.tile([128, QB], F32R, tag="pt")
                                nc.scalar.activation(pt[:], stp[:], EXP,
                                                     scale=SCALE)
                                nc.tensor.matmul(
                                    avp[:], V_b[:, ktc * 128:(ktc + 1) * 128],
                                    pt[:], start=(ktc == 0),
                                    stop=(ktc == nkt - 1))
                                nc.tensor.matmul(
                                    smp[:], ones_t[:, 0:1], pt[:],
                                    start=(ktc == 0), stop=(ktc == nkt - 1))
                            att_un = aupool.tile([128, QB], F32R, tag="au")
                            nc.scalar.copy(att_un[:], avp[:])
                            s_sb = smpool.tile([1, QB], F32, tag="ssb")
                            nc.scalar.copy(s_sb[:], smp[:])
                            r_sb = smpool.tile([1, QB], F32, tag="rsb")
                            nc.vector.reciprocal(r_sb[:], s_sb[:])
                            r_bc = accpool.tile([128, QB], F32, tag="rbc")
                            nc.gpsimd.partition_broadcast(r_bc[:], r_sb[:])
                            nc.vector.tensor_mul(
                                att_h[h][:, q0:q0 + QB], att_un[:], r_bc[:])
                # previous half's output projection, emitted here so its PE
                # work queues behind this half's attention and the
                # normalization chain never stalls the in-order PE stream
                if pending is not None:
                    emit_wo(*pending)
                pending = (att_h, b, hb)
        emit_wo(*pending)
    nc.compile()
    return nc


_program = None


def _get_program():
    global _program
    if _program is None:
        _program = _build_program()
    return _program


def kernel(**inputs) -> np.ndarray:
    x = np.asarray(inputs["x"], dtype=np.float32)
    wq = np.asarray(inputs["wq"], dtype=np.float32)
    wk = np.asarray(inputs["wk"], dtype=np.float32)
    wv = np.asarray(inputs["wv"], dtype=np.float32)
    wo = np.asarray(inputs["wo"], dtype=np.float32)
    cos = np.asarray(inputs["freqs_cos"], dtype=np.float32)
    sin = np.asarray(inputs["freqs_sin"], dtype=np.float32)
    mask = np.asarray(inputs["mask"], dtype=np.float32)
    start_pos = int(np.asarray(inputs.get("start_pos", 0)))
    assert start_pos == 0, "kernel specialized for start_pos == 0"

    # Even/odd RoPE pair split within each head's 128 features.
    perm = np.concatenate([np.arange(0, HD, 2), np.arange(1, HD, 2)])

    # x tiled: x4[gg, jb] rows = [128, 8*256] where row p, col c*256+w =
    # x_token[gg*256 + w, (jb*8+c)*128 + p]
    xT = x.reshape(T, D).T                              # [D, T]
    x4 = np.ascontiguousarray(
        xT.reshape(NJB, 8, 128, 16, NW).transpose(3, 0, 2, 1, 4)
        .reshape(16 * NJB * 128, 8 * NW)).astype(ml_dtypes.bfloat16)

    cosT = cos.T                                        # [64, S]
    sinT = sin.T
    ropc = np.ascontiguousarray(np.concatenate([cosT, cosT], axis=0))
    rops = np.ascontiguousarray(np.concatenate([-sinT, sinT], axis=0))
    masktv = [np.ascontiguousarray(
        np.maximum(mask[:QB, v * 128:(v + 1) * 128], NEG)
        .astype(np.float32).T) for v in range(4)]

    in_maps = []
    for c in range(N_CORES):
        wq_c = (wq[c * FL:(c + 1) * FL].reshape(QH, HD, D)[:, perm, :]
                .reshape(FL, D))
        wk_c = wk[c * HD:(c + 1) * HD][perm, :]
        wv_c = wv[c * HD:(c + 1) * HD]
        wo_c = wo[:, c * FL:(c + 1) * FL]
        # wq4[p, k*512 + f] = wq_c[f, k*128+p]  (k-chunk-major, bf16)
        wq4 = np.ascontiguousarray(
            wq_c.T.reshape(DKD, 128, FL).transpose(1, 0, 2)
            .reshape(128, DKD * FL)).astype(ml_dtypes.bfloat16)
        wk4 = np.ascontiguousarray(
            wk_c.T.reshape(DKD, 128, HD).transpose(1, 0, 2)
            .reshape(128, DKD * HD)).astype(ml_dtypes.bfloat16)
        wv4 = np.ascontiguousarray(
            wv_c.T.reshape(DKD, 128, HD).transpose(1, 0, 2)
            .reshape(128, DKD * HD)).astype(ml_dtypes.bfloat16)
        # wot[dg*128+p, f*512+c] = wo_c[dg*512+c, f*128+p]
        wot = np.ascontiguousarray(
            wo_c.T.reshape(QH, 128, NDG, QB).transpose(2, 1, 0, 3)
            .reshape(NDG * 128, QH * QB))
        in_maps.append({
            "x4": x4,
            "wq4": wq4,
            "wk4": wk4,
            "wv4": wv4,
            "wot": wot,
            "ropc": ropc,
            "rops": rops,
            "idin": np.eye(128, dtype=np.float32),
            "onesin": np.ones((128, 1), dtype=np.float32),
            "maskt0": masktv[0],
            "maskt1": masktv[1],
            "maskt2": masktv[2],
            "maskt3": masktv[3],
        })

    nc = _get_program()
    trace = bool(int(os.environ.get("GQA_TRACE", "0")))
    kwargs = {}
    if trace:
        tmpdir = os.environ.get("GQA_TRACE_DIR") or None
        kwargs = dict(trace=True, tmpdir=tmpdir, trace_cores=[0])
    res = run_bass_kernel_spmd(nc, in_maps, list(range(N_CORES)), **kwargs)
    kernel.last_results = res

    acc = np.zeros((T // 128, 128, D), dtype=np.float64)
    for c in range(N_CORES):
        yt = np.asarray(res.results[c]["y_t"], dtype=np.float64)
        acc += yt.reshape(T // 128, NDG, 128, QB).transpose(0, 2, 1, 3) \
                 .reshape(T // 128, 128, D)
    return acc.astype(np.float32).reshape(B, S, D)


# revision 17
# speedup vs baseline: 1.2444x; 1.0392x over previous
"""GQA attention layer (B=2, S=2048, D=4096, 32 Q heads / 8 KV heads, RoPE,
causal) on 8 Trainium2 NeuronCores, tensor-parallel over heads.

Each core owns 4 Q heads + 1 KV head and computes the whole layer for its
slice in ONE fused pass: per 1024-token half-batch it projects Q/K/V
(bf16 operands, fp32 PSUM accumulation), applies RoPE straight out of
PSUM on the vector engine (swap-free half-partition formulation), runs
causal attention out of SBUF-resident K/V, and emits a streamed output
projection deferred by one half so normalization latency never stalls
the PE.  The host sums the 8 partial outputs (bf16 on the wire).

Key layout choices:
 - activations feature-major [feature_partition, token_free]; every
   matmul contracts over the partition dim.
 - projection PSUM packs two 256-wide outputs per 2KB bank using the
   per-element has_written semantics (single start=True per bank).
 - V is transposed to token-major on the PE (deferred batch of
   transposes per half, packed 2 per PSUM bank).
 - softmax denominators: exp tiles accumulated on DVE, reduced across
   partitions + broadcast in one gpsimd partition_all_reduce.
"""

import os
import sys
import types
from contextlib import ExitStack

import numpy as np
import ml_dtypes

import concourse.bass as bass
import concourse.tile as tile
from concourse import bacc
from concourse import mybir
from concourse import bass_utils
from concourse.bass_utils import run_bass_kernel_spmd

# Optional NTFF profiling support under axon (trimmed image lacks
# antenv.axon_hooks); harmless when unavailable.
try:
    import antenv  # noqa: F401
    from trn_agent_boot.trn_boot import _ntff_profile_via_ctypes

    if "antenv.axon_hooks" not in sys.modules:
        _hooks_mod = types.ModuleType("antenv.axon_hooks")
        _hook = _ntff_profile_via_ctypes("/opt/axon/libaxon_pjrt.so")
        _hooks_mod.get_axon_ntff_profile_hook = lambda: _hook
        _hooks_mod.set_axon_ntff_profile_hook = lambda h: None
        sys.modules["antenv.axon_hooks"] = _hooks_mod
    bass_utils.upload_artifacts = lambda tmpdir: "local://skipped"
except Exception:
    pass

F32 = mybir.dt.float32
F32R = mybir.dt.float32r
BF16 = mybir.dt.bfloat16
EXP = mybir.ActivationFunctionType.Exp

B, S, D = 2, 2048, 4096
NH, NKV, HD = 32, 8, 128
T = B * S
N_CORES = 8
QH = NH // N_CORES              # 4 local q heads
FL = QH * HD                    # 512 local q features
SCALE = 1.0 / float(np.sqrt(HD))
NEG = -1.0e30

NW = 256                        # tokens per projection group
HALF = 1024                     # tokens per fused pipeline stage
NG = HALF // NW                 # 4 proj groups per half
QB = 512                        # q-block width in attention
DKD = D // 128                  # 32 contraction chunks
NJB = 4                         # x/wq chunk-groups (8 k-chunks each)
NDG = D // QB                   # 8 output-projection column blocks


def _build_program():
    nc = bacc.Bacc("TRN2", target_bir_lowering=False, debug=False,
                   num_devices=N_CORES)

    # Host-pretiled inputs (see kernel() for exact layouts).
    x4 = nc.dram_tensor("x4", [16 * NJB * 128, 8 * NW], BF16,
                        kind="ExternalInput").ap()
    wq4 = nc.dram_tensor("wq4", [128, DKD * FL], BF16, kind="ExternalInput").ap()
    wk4 = nc.dram_tensor("wk4", [128, DKD * HD], BF16, kind="ExternalInput").ap()
    wv4 = nc.dram_tensor("wv4", [128, DKD * HD], BF16, kind="ExternalInput").ap()
    wot = nc.dram_tensor("wot", [NDG * 128, QH * QB], F32R,
                         kind="ExternalInput").ap()
    ropc = nc.dram_tensor("ropc", [HD, S], F32, kind="ExternalInput").ap()
    rops = nc.dram_tensor("rops", [HD, S], F32, kind="ExternalInput").ap()
    idin = nc.dram_tensor("idin", [128, 128], F32R, kind="ExternalInput").ap()
    onesin = nc.dram_tensor("onesin", [128, 1], F32R, kind="ExternalInput").ap()
    maskt = [nc.dram_tensor(f"maskt{v}", [128, QB], F32R,
                            kind="ExternalInput").ap() for v in range(4)]
    # Output: y_t[tg, dg] = y[tg*128:(tg+1)*128, dg*512:(dg+1)*512] in bf16.
    y_t = nc.dram_tensor("y_t", [(T // 128) * NDG * 128, QB], BF16,
                         kind="ExternalOutput").ap()

    with tile.TileContext(nc) as tc, ExitStack() as ctx:
        # ------------------------------------------------------------------
        # Persistent SBUF tiles
        # ------------------------------------------------------------------
        # Weight loads first (wq chunk-group 0 before everything else so the
        # first projection matmuls can start within a few microseconds; the
        # remaining const loads are emitted later to keep early DMA
        # semaphore lanes free — lanes recycle round-robin and a DMA behind
        # a busy lane inherits its predecessor's completion wait).
        wpool = ctx.enter_context(tc.tile_pool(name="weights", bufs=1))
        wq_sb = wpool.tile([128, DKD * FL], BF16, tag="wq")
        nc.sync.dma_start(wq_sb[:, 0:8 * FL], wq4[:, 0:8 * FL])
        wk_sb = wpool.tile([128, DKD * HD], BF16, tag="wk")
        nc.sync.dma_start(wk_sb[:], wk4)
        wv_sb = wpool.tile([128, DKD * HD], BF16, tag="wv")
        nc.sync.dma_start(wv_sb[:], wv4)

        const = ctx.enter_context(tc.tile_pool(name="const", bufs=1))
        ident = const.tile([128, 128], F32R)
        ones_t = const.tile([128, 1], F32R, tag="ones")
        mtv = [const.tile([128, QB], F32R, tag=f"mtv{v}", name=f"mtv{v}")
               for v in range(4)]

        def emit_late_weight_loads():
            for jb in range(1, NJB):
                nc.sync.dma_start(
                    wq_sb[:, jb * 8 * FL:(jb + 1) * 8 * FL],
                    wq4[:, jb * 8 * FL:(jb + 1) * 8 * FL])

        def emit_const_loads():
            nc.sync.dma_start(ident[:], idin)
            nc.sync.dma_start(ones_t[:], onesin)
            for v in range(4):
                nc.sync.dma_start(mtv[v][:], maskt[v])

        rcpool = ctx.enter_context(tc.tile_pool(name="ropec", bufs=1))
        cos_s = rcpool.tile([HD, HALF], F32, tag="cos")
        sin_s = rcpool.tile([HD, HALF], F32, tag="sin")

        apool = ctx.enter_context(tc.tile_pool(name="acts", bufs=1))
        q_half = [apool.tile([128, HALF], F32R, tag=f"qh{m}", name=f"qh{m}")
                  for m in range(QH)]
        kT = apool.tile([128, S], F32R, tag="kT")
        V_b = apool.tile([128, S], F32R, tag="V_b")

        attpool = ctx.enter_context(tc.tile_pool(name="att", bufs=2))
        wopool = ctx.enter_context(tc.tile_pool(name="wostream", bufs=2))
        xpool = ctx.enter_context(tc.tile_pool(name="xin", bufs=3))
        ptpool = ctx.enter_context(tc.tile_pool(name="pt", bufs=3))
        accpool = ctx.enter_context(tc.tile_pool(name="acc", bufs=2))
        smpool = ctx.enter_context(tc.tile_pool(name="sums", bufs=2))
        aupool = ctx.enter_context(tc.tile_pool(name="attun", bufs=2))
        yspool = ctx.enter_context(tc.tile_pool(name="ystage", bufs=4))
        vstpool = ctx.enter_context(tc.tile_pool(name="vst", bufs=4))
        rtpool = ctx.enter_context(tc.tile_pool(name="ropetmp", bufs=4))
        ypsum = ctx.enter_context(tc.tile_pool(name="yps", bufs=2, space="PSUM"))

        def rope_evict(ps, out, g):
            """RoPE on the even/odd-split feature layout, swap-free:
            out[0:64]  = ps[0:64]*cos + ps[64:128]*(-sin)
            out[64:128]= ps[64:128]*cos + ps[0:64]*(+sin)
            (cos_s rows duplicated; sin_s rows = [-sin; sin]).
            Reads ps (PSUM) directly on the vector engine."""
            c0 = g * NW
            tmp = rtpool.tile([128, NW], F32, tag="rt")
            # the four PSUM-reading muls stay on DVE; the two adds go to
            # the (otherwise idle) gpsimd engine to keep DVE off the
            # critical path
            nc.vector.tensor_mul(tmp[0:64, :], ps[64:128, :],
                                 sin_s[0:64, c0:c0 + NW])
            nc.vector.tensor_mul(out[0:64, :], ps[0:64, :],
                                 cos_s[0:64, c0:c0 + NW])
            nc.vector.tensor_mul(tmp[64:128, :], ps[0:64, :],
                                 sin_s[64:128, c0:c0 + NW])
            nc.vector.tensor_mul(out[64:128, :], ps[64:128, :],
                                 cos_s[64:128, c0:c0 + NW])
            nc.gpsimd.tensor_add(out[0:64, :], out[0:64, :], tmp[0:64, :])
            nc.gpsimd.tensor_add(out[64:128, :], out[64:128, :],
                                 tmp[64:128, :])

        def emit_wo(att_h, b, hb):
            """Output projection for one 1024-token half: y-slice =
            att_h (4x[128,1024] feature-major) contracted with streamed
            wo column blocks."""
            tg0 = b * (S // 128) + hb * (HALF // 128)
            for dg in range(NDG):
                wod = wopool.tile([128, QH * QB], F32R, tag="wod")
                nc.sync.dma_start(wod[:], wot[dg * 128:(dg + 1) * 128, :])
                for tcx in range(HALF // 128):
                    yp = ypsum.tile([128, QB], F32, tag="yp")
                    for f in range(QH):
                        nc.tensor.matmul(
                            yp[:], att_h[f][:, tcx * 128:(tcx + 1) * 128],
                            wod[:, f * QB:(f + 1) * QB],
                            start=(f == 0), stop=(f == QH - 1))
                    ysb = yspool.tile([128, QB], BF16, tag="ysb")
                    nc.vector.tensor_copy(ysb[:], yp[:])
                    tg = tg0 + tcx
                    nc.sync.dma_start(
                        y_t[(tg * NDG + dg) * 128:(tg * NDG + dg + 1) * 128, :],
                        ysb[:])

        pending = None
        for b in range(B):
            for hb in range(2):
                base = hb * HALF            # position within batch
                # rope constants for this half's positions
                nc.sync.dma_start(cos_s[:], ropc[:, base:base + HALF])
                nc.sync.dma_start(sin_s[:], rops[:, base:base + HALF])

                # ----------------------------------------------------------
                # QKV projection + RoPE for this half
                # ----------------------------------------------------------
                with tc.tile_pool(name="projps", bufs=1, space="PSUM") as pps:
                    def emit_vt(g, vst):
                        """V transpose to token-major, 2 per PSUM bank."""
                        vtp = pps.tile([128, NW], F32R, tag="vtp", bufs=1)
                        nc.tensor.transpose(
                            vtp[:, 0:128], vst[:, 0:128], ident[:])
                        nc.tensor.transpose(
                            vtp[:, 128:256], vst[:, 128:256], ident[:])
                        nc.vector.tensor_copy(
                            V_b[:, base + g * NW:base + (g + 1) * NW], vtp[:])

                    vt_pending = None
                    for g in range(NG):
                        gg = (b * S + base) // NW + g   # global 256-tok group
                        qab = pps.tile([128, 2 * NW], F32, tag="qab", bufs=2)
                        qcd = pps.tile([128, 2 * NW], F32, tag="qcd", bufs=2)
                        kv = pps.tile([128, 2 * NW], F32, tag="kv", bufs=1)
                        for jb in range(NJB):
                            xg = xpool.tile([128, 8 * NW], BF16, tag="xg")
                            r0 = (gg * NJB + jb) * 128
                            nc.sync.dma_start(xg[:], x4[r0:r0 + 128, :])
                            if b == 0 and hb == 0 and g == 0 and jb == 0:
                                emit_late_weight_loads()
                            # q matmuls first, kv last: the kv bank is
                            # single-buffered, so its previous-group RoPE /
                            # copy reads get ~3.4us of q-matmul cover before
                            # the next write touches the bank.
                            for c in range(8):
                                k = jb * 8 + c
                                xs = xg[:, c * NW:(c + 1) * NW]
                                st = (k == 0)
                                sp = (k == DKD - 1)
                                nc.tensor.matmul(
                                    qab[:, 0:NW],
                                    wq_sb[:, k * FL:k * FL + 128], xs,
                                    start=st, stop=False)
                                nc.tensor.matmul(
                                    qab[:, NW:2 * NW],
                                    wq_sb[:, k * FL + 128:k * FL + 256], xs,
                                    start=False, stop=sp)
                                nc.tensor.matmul(
                                    qcd[:, 0:NW],
                                    wq_sb[:, k * FL + 256:k * FL + 384], xs,
                                    start=st, stop=False)
                                nc.tensor.matmul(
                                    qcd[:, NW:2 * NW],
                                    wq_sb[:, k * FL + 384:k * FL + 512], xs,
                                    start=False, stop=sp)
                            for c in range(8):
                                k = jb * 8 + c
                                xs = xg[:, c * NW:(c + 1) * NW]
                                st = (k == 0)
                                sp = (k == DKD - 1)
                                nc.tensor.matmul(
                                    kv[:, 0:NW],
                                    wk_sb[:, k * HD:(k + 1) * HD], xs,
                                    start=st, stop=False)
                                nc.tensor.matmul(
                                    kv[:, NW:2 * NW],
                                    wv_sb[:, k * HD:(k + 1) * HD], xs,
                                    start=False, stop=sp)
                        # evictions: k rope + v copy first (kv is bufs=1)
                        rope_evict(kv[:, 0:NW],
                                   kT[:, base + g * NW:base + (g + 1) * NW], g)
                        vst = vstpool.tile([128, NW], F32R, tag="vst")
                        nc.scalar.copy(vst[:], kv[:, NW:2 * NW])
                        # previous group's V transpose here: its DVE copy
                        # overlaps this group's matmuls (vtp is bufs=1)
                        if vt_pending is not None:
                            emit_vt(*vt_pending)
                        vt_pending = (g, vst)
                        rope_evict(qab[:, 0:NW],
                                   q_half[0][:, g * NW:(g + 1) * NW], g)
                        rope_evict(qab[:, NW:2 * NW],
                                   q_half[1][:, g * NW:(g + 1) * NW], g)
                        rope_evict(qcd[:, 0:NW],
                                   q_half[2][:, g * NW:(g + 1) * NW], g)
                        rope_evict(qcd[:, NW:2 * NW],
                                   q_half[3][:, g * NW:(g + 1) * NW], g)
                        if b == 0 and hb == 0 and g == 0:
                            emit_const_loads()
                    emit_vt(*vt_pending)

                # ----------------------------------------------------------
                # Attention for this half (q blocks of 512)
                # ----------------------------------------------------------
                att_h = [attpool.tile([128, HALF], F32R, tag=f"at{f}",
                                      name=f"at{f}") for f in range(QH)]
                with tc.tile_pool(name="attnps", bufs=1, space="PSUM") as aps:
                    for qb in range(HALF // QB):
                        q0 = qb * QB
                        nkt = (base + q0 + QB) // 128
                        for h in range(QH):
                            avp = aps.tile([128, QB], F32, tag="avp", bufs=2)
                            smp = aps.tile([1, QB], F32, tag="smp", bufs=2)
                            for ktc in range(nkt):
                                stp = aps.tile([128, QB], F32, tag="stp",
                                               bufs=2)
                                diag = ktc >= nkt - 4
                                nc.tensor.matmul(
                                    stp[:], kT[:, ktc * 128:(ktc + 1) * 128],
                                    q_half[h][:, q0:q0 + QB],
                                    start=True, stop=not diag)
                                if diag:
                                    # causal mask added on the PE: I.T @ M
                                    # accumulates M into the score bank,
                                    # keeping the chunk chain PE->ACT only
                                    nc.tensor.matmul(
                                        stp[:], ident[:],
                                        mtv[ktc - (nkt - 4)][:],
                                        start=False, stop=True)
                                pt = ptpool.tile([128, QB], F32R, tag="pt")
                                nc.scalar.activation(pt[:], stp[:], EXP,
                                                     scale=SCALE)
                                nc.tensor.matmul(
                                    avp[:], V_b[:, ktc * 128:(ktc + 1) * 128],
                                    pt[:], start=(ktc == 0),
                                    stop=(ktc == nkt - 1))
                                nc.tensor.matmul(
                                    smp[:], ones_t[:, 0:1], pt[:],
                                    start=(ktc == 0), stop=(ktc == nkt - 1))
                            att_un = aupool.tile([128, QB], F32R, tag="au")
                            nc.scalar.copy(att_un[:], avp[:])
                            s_sb = smpool.tile([1, QB], F32, tag="ssb")
                            nc.scalar.copy(s_sb[:], smp[:])
                            r_sb = smpool.tile([1, QB], F32, tag="rsb")
                            nc.vector.reciprocal_approx_fast(r_sb[:], s_sb[:])
                            r_bc = accpool.tile([128, QB], F32, tag="rbc")
                            nc.gpsimd.partition_broadcast(r_bc[:], r_sb[:])
                            nc.vector.tensor_mul(
                                att_h[h][:, q0:q0 + QB], att_un[:], r_bc[:])
                # previous half's output projection, emitted here so its PE
                # work queues behind this half's attention and the
                # normalization chain never stalls the in-order PE stream
                if pending is not None:
                    emit_wo(*pending)
                pending = (att_h, b, hb)
        emit_wo(*pending)
    nc.compile()
    return nc


_program = None


def _get_program():
    global _program
    if _program is None:
        _program = _build_program()
    return _program


def kernel(**inputs) -> np.ndarray:
    x = np.asarray(inputs["x"], dtype=np.float32)
    wq = np.asarray(inputs["wq"], dtype=np.float32)
    wk = np.asarray(inputs["wk"], dtype=np.float32)
    wv = np.asarray(inputs["wv"], dtype=np.float32)
    wo = np.asarray(inputs["wo"], dtype=np.float32)
    cos = np.asarray(inputs["freqs_cos"], dtype=np.float32)
    sin = np.asarray(inputs["freqs_sin"], dtype=np.float32)
    mask = np.asarray(inputs["mask"], dtype=np.float32)
    start_pos = int(np.asarray(inputs.get("start_pos", 0)))
    assert start_pos == 0, "kernel specialized for start_pos == 0"

    # Even/odd RoPE pair split within each head's 128 features.
    perm = np.concatenate([np.arange(0, HD, 2), np.arange(1, HD, 2)])

    # x tiled: x4[gg, jb] rows = [128, 8*256] where row p, col c*256+w =
    # x_token[gg*256 + w, (jb*8+c)*128 + p]
    xT = x.reshape(T, D).T                              # [D, T]
    x4 = np.ascontiguousarray(
        xT.reshape(NJB, 8, 128, 16, NW).transpose(3, 0, 2, 1, 4)
        .reshape(16 * NJB * 128, 8 * NW)).astype(ml_dtypes.bfloat16)

    cosT = cos.T                                        # [64, S]
    sinT = sin.T
    ropc = np.ascontiguousarray(np.concatenate([cosT, cosT], axis=0))
    rops = np.ascontiguousarray(np.concatenate([-sinT, sinT], axis=0))
    masktv = [np.ascontiguousarray(
        np.maximum(mask[:QB, v * 128:(v + 1) * 128], NEG)
        .astype(np.float32).T) for v in range(4)]

    in_maps = []
    for c in range(N_CORES):
        wq_c = (wq[c * FL:(c + 1) * FL].reshape(QH, HD, D)[:, perm, :]
                .reshape(FL, D))
        wk_c = wk[c * HD:(c + 1) * HD][perm, :]
        wv_c = wv[c * HD:(c + 1) * HD]
        wo_c = wo[:, c * FL:(c + 1) * FL]
        # wq4[p, k*512 + f] = wq_c[f, k*128+p]  (k-chunk-major, bf16)
        wq4 = np.ascontiguousarray(
            wq_c.T.reshape(DKD, 128, FL).transpose(1, 0, 2)
            .reshape(128, DKD * FL)).astype(ml_dtypes.bfloat16)
        wk4 = np.ascontiguousarray(
            wk_c.T.reshape(DKD, 128, HD).transpose(1, 0, 2)
            .reshape(128, DKD * HD)).astype(ml_dtypes.bfloat16)
        wv4 = np.ascontiguousarray(
            wv_c.T.reshape(DKD, 128, HD).transpose(1, 0, 2)
            .reshape(128, DKD * HD)).astype(ml_dtypes.bfloat16)
        # wot[dg*128+p, f*512+c] = wo_c[dg*512+c, f*128+p]
        wot = np.ascontiguousarray(
            wo_c.T.reshape(QH, 128, NDG, QB).transpose(2, 1, 0, 3)
            .reshape(NDG * 128, QH * QB))
        in_maps.append({
            "x4": x4,
            "wq4": wq4,
            "wk4": wk4,
            "wv4": wv4,
            "wot": wot,
            "ropc": ropc,
            "rops": rops,
            "idin": np.eye(128, dtype=np.float32),
            "onesin": np.ones((128, 1), dtype=np.float32),
            "maskt0": masktv[0],
            "maskt1": masktv[1],
            "maskt2": masktv[2],
            "maskt3": masktv[3],
        })

    nc = _get_program()
    trace = bool(int(os.environ.get("GQA_TRACE", "0")))
    kwargs = {}
    if trace:
        tmpdir = os.environ.get("GQA_TRACE_DIR") or None
        kwargs = dict(trace=True, tmpdir=tmpdir, trace_cores=[0])
    res = run_bass_kernel_spmd(nc, in_maps, list(range(N_CORES)), **kwargs)
    kernel.last_results = res

    acc = np.zeros((T // 128, 128, D), dtype=np.float64)
    for c in range(N_CORES):
        yt = np.asarray(res.results[c]["y_t"], dtype=np.float64)
        acc += yt.reshape(T // 128, NDG, 128, QB).transpose(0, 2, 1, 3) \
                 .reshape(T // 128, 128, D)
    return acc.astype(np.float32).reshape(B, S, D)
